# revision 11
# baseline (speedup 1.0000x reference)
"""CARAFE upsampling kernel for 8 Trainium2 NeuronCores.

Reference op (per batch b):
  xc   = conv1x1(x, w1) + b1                     # (CC=64, H, W)
  mask = conv3x3(xc, w2, pad=1) + b2             # (100, H, W)
  mask = softmax over the 25 kernel taps (per q in 4 = SF*SF groups)
  out[q, c, h, w] = sum_k mask[q, k, h, w] * x[c, h+di-2, w+dj-2]
  out pixel-shuffled by SF=2 -> (C, 2H, 2W)

Sharding: 8 shards = batch(4) x H-halves(2).

Combine strategy (channel-major, wide bf16 ops in DVE 2x mode): per
(tap k, quadrant q) the normalized mask row [2048 px] is partition-
broadcast to a [128, 2048] bf16 tile via a stride-0 DRAM-source DMA
(runs on the DMA engines, off the compute path). The 25-tap x 4q x 2ch
accumulation then runs as [128, 2048] ops on balanced lanes:
  - PAIR: DVE tensor_mul + DVE tensor_add (both bf16 2x, ~1.2 us each)
  - XPOOL: DVE tensor_mul -> GPSIMD tensor_add (~4.2 us)
Each (q, ch) keeps one accumulator per adding engine; partials are
merged on DVE at the end and written out in bf16.
"""

import os
from functools import lru_cache

import numpy as np

import concourse.bass as bass
import concourse.mybir as mybir
from concourse import bacc
import concourse.tile as tile
from concourse.bass_utils import run_bass_kernel_spmd

F32 = mybir.dt.float32
BF16 = mybir.dt.bfloat16
import ml_dtypes as _mld

_BF16NP = _mld.bfloat16

B, C, H, W = 4, 256, 64, 64
CC = 64
SF = 2
K5 = 5
KA = K5 * K5
NQ = SF * SF
NM = NQ * KA

HL = 32
HP = HL + 4
WP = W + 4
NPIX = HL * W
NPADPIX = HP * WP

N_CORES = 8

# measured per-[128,2048]-op engine costs (ns) for lane balancing
_DVE_MUL = 1250.0
_DVE_ADD = 1210.0
_POOL_ADD = 4200.0


def _build_program():
    nc = bacc.Bacc("TRN2", target_bir_lowering=False, debug=False)

    x0_d = nc.dram_tensor("x0", [128, HP, WP], F32, kind="ExternalInput")
    x1_d = nc.dram_tensor("x1", [128, HP, WP], F32, kind="ExternalInput")
    xb0_d = nc.dram_tensor("xb0", [128, HP, WP], BF16, kind="ExternalInput")
    xb1_d = nc.dram_tensor("xb1", [128, HP, WP], BF16, kind="ExternalInput")
    w1t_d = nc.dram_tensor("w1t", [2, 128, CC], F32, kind="ExternalInput")
    w2t_d = nc.dram_tensor("w2t", [CC, 9, NM], F32, kind="ExternalInput")
    b1_d = nc.dram_tensor("b1v", [CC, 1], F32, kind="ExternalInput")
    b2_d = nc.dram_tensor("b2v", [NM, 1], F32, kind="ExternalInput")
    osum_d = nc.dram_tensor("osum", [NM, NQ], F32, kind="ExternalInput")
    orep_d = nc.dram_tensor("orep", [NQ, NM], F32, kind="ExternalInput")
    # normalized-mask staging in DRAM for stride-0 broadcast reads
    msk_d = nc.dram_tensor("mskd", [NM, NPIX], BF16, kind="Internal")

    out_d = nc.dram_tensor("out", [2, 128, NQ, NPIX], BF16, kind="ExternalOutput")

    with tile.TileContext(nc) as tc:
        with (
            tc.tile_pool(name="xpool", bufs=1) as xpool,
            tc.tile_pool(name="wpool", bufs=1) as wpool,
            tc.tile_pool(name="mpool", bufs=1) as mpool,
            tc.tile_pool(name="acc", bufs=1) as accpool,
            tc.tile_pool(name="bcast", bufs=6) as bcpool,
            tc.tile_pool(name="prod", bufs=6) as prpool,
            tc.tile_pool(name="psum", bufs=2, space="PSUM") as psum,
        ):
            # ---- load inputs -------------------------------------------
            x0 = xpool.tile([128, HP, WP], F32)
            x1 = xpool.tile([128, HP, WP], F32)
            nc.sync.dma_start(x0[:], x0_d[:])
            nc.sync.dma_start(x1[:], x1_d[:])
            xb0 = xpool.tile([128, HP, WP], BF16, tag="xb0")
            xb1 = xpool.tile([128, HP, WP], BF16, tag="xb1")
            nc.sync.dma_start(xb0[:], xb0_d[:])
            nc.sync.dma_start(xb1[:], xb1_d[:])

            w1sb = wpool.tile([128, 2, CC], F32, tag="w1sb")
            nc.sync.dma_start(w1sb[:, 0, :], w1t_d[0])
            nc.sync.dma_start(w1sb[:, 1, :], w1t_d[1])
            w2sb = wpool.tile([CC, 9, NM], F32, tag="w2sb")
            nc.sync.dma_start(w2sb[:], w2t_d[:])
            b1c = wpool.tile([CC, 1], F32, tag="b1c")
            nc.sync.dma_start(b1c[:], b1_d[:])
            b2c = wpool.tile([NM, 1], F32, tag="b2c")
            nc.sync.dma_start(b2c[:], b2_d[:])
            osum = wpool.tile([NM, NQ], F32, tag="osum")
            nc.sync.dma_start(osum[:], osum_d[:])
            orep = wpool.tile([NQ, NM], F32, tag="orep")
            nc.sync.dma_start(orep[:], orep_d[:])

            # ---- PE fences ---------------------------------------------
            for fap in (
                x0[:, 0, 0:1], x1[:, 0, 0:1], w1sb[:, 0, 0:1],
                w2sb[:, 0, 0:1], osum[:, 0:1], orep[:, 0:1],
            ):
                psf = psum.tile([1, 1], F32, tag="psf")
                nc.tensor.matmul(psf[:], fap, fap, start=True, stop=True)

            # ---- stage A: conv1x1 --------------------------------------
            xc = mpool.tile([CC, HP, WP], F32, tag="xc")
            xc_flat = xc[:].rearrange("c h w -> c (h w)")
            x0_flat = x0[:].rearrange("c h w -> c (h w)")
            x1_flat = x1[:].rearrange("c h w -> c (h w)")
            CHUNK = 512
            nchunks = (NPADPIX + CHUNK - 1) // CHUNK
            for i in range(nchunks):
                n0 = i * CHUNK
                n1 = min(NPADPIX, n0 + CHUNK)
                ps = psum.tile([CC, CHUNK], F32, tag="ps")
                nc.tensor.matmul(
                    ps[:, : n1 - n0], w1sb[:, 0, :], x0_flat[:, n0:n1],
                    start=True, stop=False,
                )
                nc.tensor.matmul(
                    ps[:, : n1 - n0], w1sb[:, 1, :], x1_flat[:, n0:n1],
                    start=False, stop=True,
                )
                nc.vector.tensor_scalar_add(
                    xc_flat[:, n0:n1], ps[:, : n1 - n0], b1c[:, 0:1]
                )

            # ---- stage B: conv3x3 -> exp -------------------------------
            msk_e = mpool.tile([NM, NPIX], F32, tag="msk_e")
            HROWS = 8
            for i in range(HL // HROWS):
                psm = psum.tile([NM, HROWS, W], F32, tag="ps")
                for tap in range(9):
                    dy, dx = tap // 3, tap % 3
                    rhs = xc[:, i * HROWS + 1 + dy : i * HROWS + 1 + dy + HROWS,
                             1 + dx : 1 + dx + W]
                    nc.tensor.matmul(
                        psm[:], w2sb[:, tap, :], rhs,
                        start=(tap == 0), stop=(tap == 8),
                    )
                me = msk_e[:].rearrange("m (h w) -> m h w", w=W)
                nc.scalar.activation(
                    me[:, i * HROWS : (i + 1) * HROWS, :], psm[:],
                    mybir.ActivationFunctionType.Exp, bias=b2c[:, 0:1],
                )

            # ---- stage C: normalize (bf16) + stage to DRAM -------------
            rs = mpool.tile([NQ, NPIX], F32, tag="rs")
            msk_nb = mpool.tile([NM, NPIX], BF16, tag="msk_nb")
            for i in range(NPIX // CHUNK):
                pss = psum.tile([NQ, CHUNK], F32, tag="ps")
                nc.tensor.matmul(
                    pss[:], osum[:], msk_e[:, i * CHUNK : (i + 1) * CHUNK],
                    start=True, stop=True,
                )
                nc.vector.reciprocal(rs[:, i * CHUNK : (i + 1) * CHUNK], pss[:])
                psr = psum.tile([NM, CHUNK], F32, tag="ps")
                nc.tensor.matmul(
                    psr[:], orep[:], rs[:, i * CHUNK : (i + 1) * CHUNK],
                    start=True, stop=True,
                )
                nc.vector.tensor_mul(
                    msk_nb[:, i * CHUNK : (i + 1) * CHUNK],
                    msk_e[:, i * CHUNK : (i + 1) * CHUNK], psr[:],
                )
                nc.sync.dma_start(
                    msk_d[:, i * CHUNK : (i + 1) * CHUNK],
                    msk_nb[:, i * CHUNK : (i + 1) * CHUNK],
                )

            # ---- stage D: combine --------------------------------------
            # lane choice: simulate both lane clocks, pick the option that
            # minimizes the max lane clock (converges to the LP split of
            # ~109 DVE-pair / 91 pool-add units).
            xbs = (xb0, xb1)
            dve_t = pool_t = 0.0
            unit_idx = 0
            for q in range(NQ):
                acc_d = {}
                acc_p = {}
                for k in range(KA):
                    di, dj = k // K5, k % K5
                    row = q * KA + k
                    bc = bcpool.tile([128, NPIX], BF16, tag="bc")
                    nc.sync.dma_start(
                        bc[:], msk_d[row : row + 1, :].broadcast_to((128, NPIX))
                    )
                    bcv = bc[:].rearrange("p (h w) -> p h w", w=W)
                    for ch in range(2):
                        win = xbs[ch][:, di : di + HL, dj : dj + W]
                        # deterministic LP split: 91 of 200 units to XPOOL
                        u = unit_idx
                        unit_idx += 1
                        use_pair = (u * 91) // 200 == ((u + 1) * 91) // 200
                        if use_pair:
                            if ch not in acc_d:
                                a = accpool.tile(
                                    [128, NPIX], BF16, tag=f"acc_d{ch}", bufs=2
                                )
                                acc_d[ch] = a
                                av = a[:].rearrange("p (h w) -> p h w", w=W)
                                nc.vector.tensor_mul(av, win, bcv)
                                dve_t += _DVE_MUL
                            else:
                                a = acc_d[ch]
                                pr = prpool.tile([128, NPIX], BF16, tag="prd")
                                prv = pr[:].rearrange("p (h w) -> p h w", w=W)
                                nc.vector.tensor_mul(prv, win, bcv)
                                nc.vector.tensor_add(a[:], a[:], pr[:])
                                dve_t += _DVE_MUL + _DVE_ADD
                        else:
                            if ch not in acc_p:
                                a = accpool.tile(
                                    [128, NPIX], BF16, tag=f"acc_p{ch}", bufs=2
                                )
                                acc_p[ch] = a
                                av = a[:].rearrange("p (h w) -> p h w", w=W)
                                nc.vector.tensor_mul(av, win, bcv)
                                dve_t += _DVE_MUL
                            else:
                                a = acc_p[ch]
                                pr = prpool.tile([128, NPIX], BF16, tag="prp")
                                prv = pr[:].rearrange("p (h w) -> p h w", w=W)
                                nc.vector.tensor_mul(prv, win, bcv)
                                nc.gpsimd.tensor_add(a[:], a[:], pr[:])
                                dve_t += _DVE_MUL
                                pool_t = max(pool_t, dve_t) + _POOL_ADD
                # merge this q's partials + write out (overlaps next q)
                for ch in range(2):
                    if ch in acc_d and ch in acc_p:
                        a = acc_d[ch]
                        nc.vector.tensor_add(a[:], a[:], acc_p[ch][:])
                        dve_t += _DVE_ADD
                    else:
                        a = acc_d.get(ch, acc_p.get(ch))
                    nc.sync.dma_start(out_d[ch, :, q, :], a[:])

    nc.compile()
    return nc


@lru_cache(maxsize=2)
def _get_program(trace_debug: bool = False):
    return _build_program()


def _host_prep(x, w1, b1, w2, b2):
    x = np.asarray(x, np.float32)
    w1 = np.asarray(w1, np.float32)
    b1 = np.asarray(b1, np.float32).reshape(CC, 1)
    w2 = np.asarray(w2, np.float32)
    b2 = np.asarray(b2, np.float32).reshape(NM, 1)

    w1t = np.ascontiguousarray(w1[:, :, 0, 0].T.reshape(2, 128, CC))
    w2t = np.ascontiguousarray(w2.transpose(1, 2, 3, 0).reshape(CC, 9, NM))
    osum = np.zeros((NM, NQ), np.float32)
    for q in range(NQ):
        osum[q * KA : (q + 1) * KA, q] = 1.0
    orep = np.ascontiguousarray(osum.T)

    in_maps = []
    for s in range(N_CORES):
        b, hh = s // 2, s % 2
        h0 = hh * HL
        xpad = np.zeros((C, HP, WP), np.float32)
        r0 = max(0, h0 - 2)
        r1 = min(H, h0 + HL + 2)
        xpad[:, (r0 - h0 + 2) : (r1 - h0 + 2), 2 : 2 + W] = x[b, :, r0:r1, :]
        xb = xpad.astype(_BF16NP)
        in_maps.append(
            {
                "x0": np.ascontiguousarray(xpad[:128]),
                "x1": np.ascontiguousarray(xpad[128:]),
                "xb0": np.ascontiguousarray(xb[:128]),
                "xb1": np.ascontiguousarray(xb[128:]),
                "w1t": w1t,
                "w2t": w2t,
                "b1v": b1,
                "b2v": b2,
                "osum": osum,
                "orep": orep,
            }
        )
    return in_maps


def _host_post(results):
    out = np.empty((B, C, H * SF, W * SF), np.float32)
    for s in range(N_CORES):
        b, hh = s // 2, s % 2
        o = np.asarray(results[s]["out"], np.float32)  # [2, 128, NQ, NPIX]
        o = o.reshape(2, 128, NQ, HL, W).reshape(C, SF, SF, HL, W)
        o = o.transpose(0, 3, 1, 4, 2).reshape(C, HL * SF, W * SF)
        out[b, :, hh * HL * SF : (hh + 1) * HL * SF, :] = o
    return out


def kernel(x, w1, b1, w2, b2):
    nc = _get_program(False)
    in_maps = _host_prep(x, w1, b1, w2, b2)
    res = run_bass_kernel_spmd(nc, in_maps, list(range(N_CORES)))
    return _host_post(res.results)


# revision 17
# speedup vs baseline: 1.7931x; 1.7931x over previous
"""CARAFE upsampling kernel for 8 Trainium2 NeuronCores.

Reference op (per batch b):
  xc   = conv1x1(x, w1) + b1                     # (CC=64, H, W)
  mask = conv3x3(xc, w2, pad=1) + b2             # (100, H, W)
  mask = softmax over the 25 kernel taps (per q in 4 = SF*SF groups)
  out[q, c, h, w] = sum_k mask[q, k, h, w] * x[c, h+di-2, w+dj-2]
  out pixel-shuffled by SF=2 -> (C, 2H, 2W)

Sharding: 8 shards = batch(4) x H-halves(2).

Combine strategy (channel-major, wide bf16 ops in DVE 2x mode): per
(tap k, quadrant q) the normalized mask row [2048 px] is partition-
broadcast to a [128, 2048] bf16 tile via a stride-0 DRAM-source DMA
(runs on the DMA engines, off the compute path). The 25-tap x 4q x 2ch
accumulation then runs as [128, 2048] ops on balanced lanes:
  - PAIR: DVE tensor_mul + DVE tensor_add (both bf16 2x, ~1.2 us each)
  - XPOOL: DVE tensor_mul -> GPSIMD tensor_add (~4.2 us)
Each (q, ch) keeps one accumulator per adding engine; partials are
merged on DVE at the end and written out in bf16.
"""

import os
from functools import lru_cache

import numpy as np

import concourse.bass as bass
import concourse.mybir as mybir
from concourse import bacc
import concourse.tile as tile
from concourse.bass_utils import run_bass_kernel_spmd

F32 = mybir.dt.float32
BF16 = mybir.dt.bfloat16
import ml_dtypes as _mld

_BF16NP = _mld.bfloat16

B, C, H, W = 4, 256, 64, 64
CC = 64
SF = 2
K5 = 5
KA = K5 * K5
NQ = SF * SF
NM = NQ * KA

HL = 32
HP = HL + 4
WP = W + 4
NPIX = HL * W
NPADPIX = HP * WP

N_CORES = 8

# measured per-[128,2048]-op engine costs (ns) for lane balancing
_DVE_MUL = 1250.0
_DVE_ADD = 1210.0
# PE identity-matmul add: 4x ([128,512] matmul + ldweights) per unit
_PE_ADD = 2300.0
# DVE-add units out of 200 (rest accumulate on PE via PSUM)
_N_DVE_ADD = 89


def _build_program():
    nc = bacc.Bacc("TRN2", target_bir_lowering=False, debug=False)

    x0_d = nc.dram_tensor("x0", [128, HP, WP], F32, kind="ExternalInput")
    x1_d = nc.dram_tensor("x1", [128, HP, WP], F32, kind="ExternalInput")
    xb0_d = nc.dram_tensor("xb0", [128, HP, WP], BF16, kind="ExternalInput")
    xb1_d = nc.dram_tensor("xb1", [128, HP, WP], BF16, kind="ExternalInput")
    w1t_d = nc.dram_tensor("w1t", [2, 128, CC], F32, kind="ExternalInput")
    w2t_d = nc.dram_tensor("w2t", [CC, 9, NM], F32, kind="ExternalInput")
    b1_d = nc.dram_tensor("b1v", [CC, 1], F32, kind="ExternalInput")
    b2_d = nc.dram_tensor("b2v", [NM, 1], F32, kind="ExternalInput")
    osum_d = nc.dram_tensor("osum", [NM, NQ], F32, kind="ExternalInput")
    orep_d = nc.dram_tensor("orep", [NQ, NM], F32, kind="ExternalInput")
    i128_d = nc.dram_tensor("i128", [128, 128], BF16, kind="ExternalInput")
    # normalized-mask staging in DRAM for stride-0 broadcast reads
    msk_d = nc.dram_tensor("mskd", [NM, NPIX], BF16, kind="Internal")

    out_d = nc.dram_tensor("out", [2, 128, NQ, NPIX], BF16, kind="ExternalOutput")

    with tile.TileContext(nc) as tc:
        with (
            tc.tile_pool(name="xpool", bufs=1) as xpool,
            tc.tile_pool(name="wpool", bufs=1) as wpool,
            tc.tile_pool(name="mpool", bufs=1) as mpool,
            tc.tile_pool(name="acc", bufs=1) as accpool,
            tc.tile_pool(name="bcast", bufs=6) as bcpool,
            tc.tile_pool(name="prod", bufs=6) as prpool,
            tc.tile_pool(name="ostage", bufs=4) as outpool,
        ):
            # ---- load inputs -------------------------------------------
            x0 = xpool.tile([128, HP, WP], F32)
            x1 = xpool.tile([128, HP, WP], F32)
            nc.sync.dma_start(x0[:], x0_d[:])
            nc.sync.dma_start(x1[:], x1_d[:])
            xb0 = xpool.tile([128, HP, WP], BF16, tag="xb0")
            xb1 = xpool.tile([128, HP, WP], BF16, tag="xb1")
            nc.sync.dma_start(xb0[:], xb0_d[:])
            nc.sync.dma_start(xb1[:], xb1_d[:])

            w1sb = wpool.tile([128, 2, CC], F32, tag="w1sb")
            nc.sync.dma_start(w1sb[:, 0, :], w1t_d[0])
            nc.sync.dma_start(w1sb[:, 1, :], w1t_d[1])
            w2sb = wpool.tile([CC, 9, NM], F32, tag="w2sb")
            nc.sync.dma_start(w2sb[:], w2t_d[:])
            b1c = wpool.tile([CC, 1], F32, tag="b1c")
            nc.sync.dma_start(b1c[:], b1_d[:])
            b2c = wpool.tile([NM, 1], F32, tag="b2c")
            nc.sync.dma_start(b2c[:], b2_d[:])
            osum = wpool.tile([NM, NQ], F32, tag="osum")
            nc.sync.dma_start(osum[:], osum_d[:])
            orep = wpool.tile([NQ, NM], F32, tag="orep")
            nc.sync.dma_start(orep[:], orep_d[:])

            i128 = wpool.tile([128, 128], BF16, tag="i128")
            nc.sync.dma_start(i128[:], i128_d[:])

            with tc.tile_pool(name="psum_head", bufs=2, space="PSUM") as psum:
                # ---- PE fences -----------------------------------------
                for fap in (
                    x0[:, 0, 0:1], x1[:, 0, 0:1], w1sb[:, 0, 0:1],
                    w2sb[:, 0, 0:1], osum[:, 0:1], orep[:, 0:1],
                    i128[:, 0:1],
                ):
                    psf = psum.tile([1, 1], F32, tag="psf")
                    nc.tensor.matmul(psf[:], fap, fap, start=True, stop=True)

                # ---- stage A: conv1x1 ----------------------------------
                xc = mpool.tile([CC, HP, WP], F32, tag="xc")
                xc_flat = xc[:].rearrange("c h w -> c (h w)")
                x0_flat = x0[:].rearrange("c h w -> c (h w)")
                x1_flat = x1[:].rearrange("c h w -> c (h w)")
                CHUNK = 512
                nchunks = (NPADPIX + CHUNK - 1) // CHUNK
                for i in range(nchunks):
                    n0 = i * CHUNK
                    n1 = min(NPADPIX, n0 + CHUNK)
                    ps = psum.tile([CC, CHUNK], F32, tag="ps")
                    nc.tensor.matmul(
                        ps[:, : n1 - n0], w1sb[:, 0, :], x0_flat[:, n0:n1],
                        start=True, stop=False,
                    )
                    nc.tensor.matmul(
                        ps[:, : n1 - n0], w1sb[:, 1, :], x1_flat[:, n0:n1],
                        start=False, stop=True,
                    )
                    nc.vector.tensor_scalar_add(
                        xc_flat[:, n0:n1], ps[:, : n1 - n0], b1c[:, 0:1]
                    )

                # ---- stage B: conv3x3 -> exp ---------------------------
                msk_e = mpool.tile([NM, NPIX], F32, tag="msk_e")
                HROWS = 8
                for i in range(HL // HROWS):
                    psm = psum.tile([NM, HROWS, W], F32, tag="ps")
                    for tap in range(9):
                        dy, dx = tap // 3, tap % 3
                        rhs = xc[:, i * HROWS + 1 + dy : i * HROWS + 1 + dy + HROWS,
                                 1 + dx : 1 + dx + W]
                        nc.tensor.matmul(
                            psm[:], w2sb[:, tap, :], rhs,
                            start=(tap == 0), stop=(tap == 8),
                        )
                    me = msk_e[:].rearrange("m (h w) -> m h w", w=W)
                    nc.scalar.activation(
                        me[:, i * HROWS : (i + 1) * HROWS, :], psm[:],
                        mybir.ActivationFunctionType.Exp, bias=b2c[:, 0:1],
                    )

                # ---- stage C: normalize (bf16) + stage to DRAM ---------
                rs = mpool.tile([NQ, NPIX], F32, tag="rs")
                msk_nb = mpool.tile([NM, NPIX], BF16, tag="msk_nb")
                for i in range(NPIX // CHUNK):
                    pss = psum.tile([NQ, CHUNK], F32, tag="ps")
                    nc.tensor.matmul(
                        pss[:], osum[:], msk_e[:, i * CHUNK : (i + 1) * CHUNK],
                        start=True, stop=True,
                    )
                    nc.vector.reciprocal(
                        rs[:, i * CHUNK : (i + 1) * CHUNK], pss[:]
                    )
                    psr = psum.tile([NM, CHUNK], F32, tag="ps")
                    nc.tensor.matmul(
                        psr[:], orep[:], rs[:, i * CHUNK : (i + 1) * CHUNK],
                        start=True, stop=True,
                    )
                    nc.vector.tensor_mul(
                        msk_nb[:, i * CHUNK : (i + 1) * CHUNK],
                        msk_e[:, i * CHUNK : (i + 1) * CHUNK], psr[:],
                    )
                    nc.sync.dma_start(
                        msk_d[:, i * CHUNK : (i + 1) * CHUNK],
                        msk_nb[:, i * CHUNK : (i + 1) * CHUNK],
                    )

            # ---- stage D: combine --------------------------------------
            # DVE does every mul; adds split between DVE tensor_add and
            # PE identity-matmul accumulation into PSUM (4 banks per ch).
            CH4 = NPIX // 4  # 512, one PSUM bank
            with tc.tile_pool(name="psum_acc", bufs=1, space="PSUM") as psacc:
                xbs = (xb0, xb1)
                unit_idx = 0
                for q in range(NQ):
                    # precompute lane per (k, ch) for this q
                    lane = {}
                    pe_ks = {0: [], 1: []}
                    for k in range(KA):
                        for ch in range(2):
                            u = unit_idx
                            unit_idx += 1
                            use_dve = (u * _N_DVE_ADD) // 200 != (
                                (u + 1) * _N_DVE_ADD
                            ) // 200
                            lane[(k, ch)] = use_dve
                            if not use_dve:
                                pe_ks[ch].append(k)
                    acc_d = {}
                    acc_e = {}
                    for k in range(KA):
                        di, dj = k // K5, k % K5
                        row = q * KA + k
                        bc = bcpool.tile([128, NPIX], BF16, tag="bc")
                        nc.sync.dma_start(
                            bc[:],
                            msk_d[row : row + 1, :].broadcast_to((128, NPIX)),
                        )
                        bcv = bc[:].rearrange("p (h w) -> p h w", w=W)
                        for ch in range(2):
                            win = xbs[ch][:, di : di + HL, dj : dj + W]
                            if lane[(k, ch)]:
                                if ch not in acc_d:
                                    a = accpool.tile(
                                        [128, NPIX], BF16,
                                        tag=f"acc_d{ch}", bufs=2,
                                    )
                                    acc_d[ch] = a
                                    av = a[:].rearrange(
                                        "p (h w) -> p h w", w=W
                                    )
                                    nc.vector.tensor_mul(av, win, bcv)
                                else:
                                    a = acc_d[ch]
                                    pr = prpool.tile(
                                        [128, NPIX], BF16, tag="prd"
                                    )
                                    prv = pr[:].rearrange(
                                        "p (h w) -> p h w", w=W
                                    )
                                    nc.vector.tensor_mul(prv, win, bcv)
                                    nc.vector.tensor_add(a[:], a[:], pr[:])
                            else:
                                pr = prpool.tile(
                                    [128, NPIX], BF16, tag="prp"
                                )
                                prv = pr[:].rearrange("p (h w) -> p h w", w=W)
                                nc.vector.tensor_mul(prv, win, bcv)
                                first = k == pe_ks[ch][0]
                                last = k == pe_ks[ch][-1]
                                if first:
                                    acc_e[ch] = [
                                        psacc.tile(
                                            [128, CH4], F32,
                                            tag=f"pacc{ch}_{cc}",
                                            name=f"pacc{ch}_{cc}",
                                        )
                                        for cc in range(4)
                                    ]
                                for cc in range(4):
                                    nc.tensor.matmul(
                                        acc_e[ch][cc][:],
                                        i128[:],
                                        pr[:, cc * CH4 : (cc + 1) * CH4],
                                        start=first,
                                        stop=last,
                                    )
                    # ---- drain this q: copy PSUM partials, merge, DMA out
                    for ch in range(2):
                        parts = []
                        if ch in acc_e:
                            pe_sb = outpool.tile(
                                [128, NPIX], BF16, tag="pe_sb"
                            )
                            for cc in range(4):
                                nc.scalar.copy(
                                    pe_sb[:, cc * CH4 : (cc + 1) * CH4],
                                    acc_e[ch][cc][:],
                                )
                            parts.append(pe_sb)
                        if ch in acc_d:
                            parts.append(acc_d[ch])
                        if len(parts) == 2:
                            ost = outpool.tile([128, NPIX], BF16, tag="ost")
                            nc.vector.tensor_add(
                                ost[:], parts[0][:], parts[1][:]
                            )
                        else:
                            ost = parts[0]
                        nc.sync.dma_start(out_d[ch, :, q, :], ost[:])

    nc.compile()
    return nc


@lru_cache(maxsize=2)
def _get_program(trace_debug: bool = False):
    return _build_program()


def _host_prep(x, w1, b1, w2, b2):
    x = np.asarray(x, np.float32)
    w1 = np.asarray(w1, np.float32)
    b1 = np.asarray(b1, np.float32).reshape(CC, 1)
    w2 = np.asarray(w2, np.float32)
    b2 = np.asarray(b2, np.float32).reshape(NM, 1)

    w1t = np.ascontiguousarray(w1[:, :, 0, 0].T.reshape(2, 128, CC))
    w2t = np.ascontiguousarray(w2.transpose(1, 2, 3, 0).reshape(CC, 9, NM))
    osum = np.zeros((NM, NQ), np.float32)
    for q in range(NQ):
        osum[q * KA : (q + 1) * KA, q] = 1.0
    orep = np.ascontiguousarray(osum.T)
    i128 = np.eye(128, dtype=np.float32).astype(_BF16NP)

    in_maps = []
    for s in range(N_CORES):
        b, hh = s // 2, s % 2
        h0 = hh * HL
        xpad = np.zeros((C, HP, WP), np.float32)
        r0 = max(0, h0 - 2)
        r1 = min(H, h0 + HL + 2)
        xpad[:, (r0 - h0 + 2) : (r1 - h0 + 2), 2 : 2 + W] = x[b, :, r0:r1, :]
        xb = xpad.astype(_BF16NP)
        in_maps.append(
            {
                "x0": np.ascontiguousarray(xpad[:128]),
                "x1": np.ascontiguousarray(xpad[128:]),
                "xb0": np.ascontiguousarray(xb[:128]),
                "xb1": np.ascontiguousarray(xb[128:]),
                "w1t": w1t,
                "w2t": w2t,
                "b1v": b1,
                "b2v": b2,
                "osum": osum,
                "orep": orep,
                "i128": i128,
            }
        )
    return in_maps


def _host_post(results):
    out = np.empty((B, C, H * SF, W * SF), np.float32)
    for s in range(N_CORES):
        b, hh = s // 2, s % 2
        o = np.asarray(results[s]["out"], np.float32)  # [2, 128, NQ, NPIX]
        o = o.reshape(2, 128, NQ, HL, W).reshape(C, SF, SF, HL, W)
        o = o.transpose(0, 3, 1, 4, 2).reshape(C, HL * SF, W * SF)
        out[b, :, hh * HL * SF : (hh + 1) * HL * SF, :] = o
    return out


def kernel(x, w1, b1, w2, b2):
    nc = _get_program(False)
    in_maps = _host_prep(x, w1, b1, w2, b2)
    res = run_bass_kernel_spmd(nc, in_maps, list(range(N_CORES)))
    return _host_post(res.results)


# revision 18
# speedup vs baseline: 1.8851x; 1.0513x over previous
"""CARAFE upsampling kernel for 8 Trainium2 NeuronCores.

Reference op (per batch b):
  xc   = conv1x1(x, w1) + b1                     # (CC=64, H, W)
  mask = conv3x3(xc, w2, pad=1) + b2             # (100, H, W)
  mask = softmax over the 25 kernel taps (per q in 4 = SF*SF groups)
  out[q, c, h, w] = sum_k mask[q, k, h, w] * x[c, h+di-2, w+dj-2]
  out pixel-shuffled by SF=2 -> (C, 2H, 2W)

Sharding: 8 shards = batch(4) x H-halves(2).

Combine strategy (channel-major, wide bf16 ops in DVE 2x mode): per
(tap k, quadrant q) the normalized mask row [2048 px] is partition-
broadcast to a [128, 2048] bf16 tile via a stride-0 DRAM-source DMA
(runs on the DMA engines, off the compute path). The 25-tap x 4q x 2ch
accumulation then runs as [128, 2048] ops on balanced lanes:
  - PAIR: DVE tensor_mul + DVE tensor_add (both bf16 2x, ~1.2 us each)
  - XPOOL: DVE tensor_mul -> GPSIMD tensor_add (~4.2 us)
Each (q, ch) keeps one accumulator per adding engine; partials are
merged on DVE at the end and written out in bf16.
"""

import os
from functools import lru_cache

import numpy as np

import concourse.bass as bass
import concourse.mybir as mybir
from concourse import bacc
import concourse.tile as tile
from concourse.bass_utils import run_bass_kernel_spmd

F32 = mybir.dt.float32
BF16 = mybir.dt.bfloat16
import ml_dtypes as _mld

_BF16NP = _mld.bfloat16

B, C, H, W = 4, 256, 64, 64
CC = 64
SF = 2
K5 = 5
KA = K5 * K5
NQ = SF * SF
NM = NQ * KA

HL = 32
HP = HL + 4
WP = W + 4
NPIX = HL * W
NPADPIX = HP * WP

N_CORES = 8

# measured per-[128,2048]-op engine costs (ns) for lane balancing
_DVE_MUL = 1250.0
_DVE_ADD = 1210.0
# PE identity-matmul add: 4x ([128,512] matmul + ldweights) per unit
_PE_ADD = 2300.0
# DVE-add units out of 200 (rest accumulate on PE via PSUM)
_N_DVE_ADD = 93


def _build_program():
    nc = bacc.Bacc("TRN2", target_bir_lowering=False, debug=False)

    xb0_d = nc.dram_tensor("xb0", [128, HP, WP], BF16, kind="ExternalInput")
    xb1_d = nc.dram_tensor("xb1", [128, HP, WP], BF16, kind="ExternalInput")
    w1t_d = nc.dram_tensor("w1t", [2, 128, CC], BF16, kind="ExternalInput")
    w2t_d = nc.dram_tensor("w2t", [CC, 9, NM], BF16, kind="ExternalInput")
    b1_d = nc.dram_tensor("b1v", [CC, 1], F32, kind="ExternalInput")
    b2_d = nc.dram_tensor("b2v", [NM, 1], F32, kind="ExternalInput")
    osum_d = nc.dram_tensor("osum", [NM, NQ], F32, kind="ExternalInput")
    orep_d = nc.dram_tensor("orep", [NQ, NM], F32, kind="ExternalInput")
    i128_d = nc.dram_tensor("i128", [128, 128], BF16, kind="ExternalInput")
    # normalized-mask staging in DRAM for stride-0 broadcast reads
    msk_d = nc.dram_tensor("mskd", [NM, NPIX], BF16, kind="Internal")

    out_d = nc.dram_tensor("out", [2, 128, NQ, NPIX], BF16, kind="ExternalOutput")

    with tile.TileContext(nc) as tc:
        with (
            tc.tile_pool(name="xpool", bufs=1) as xpool,
            tc.tile_pool(name="wpool", bufs=1) as wpool,
            tc.tile_pool(name="mpool", bufs=1) as mpool,
            tc.tile_pool(name="acc", bufs=1) as accpool,
            tc.tile_pool(name="bcast", bufs=6) as bcpool,
            tc.tile_pool(name="prod", bufs=6) as prpool,
            tc.tile_pool(name="ostage", bufs=4) as outpool,
        ):
            # ---- load inputs -------------------------------------------
            xb0 = xpool.tile([128, HP, WP], BF16, tag="xb0")
            xb1 = xpool.tile([128, HP, WP], BF16, tag="xb1")
            nc.sync.dma_start(xb0[:], xb0_d[:])
            nc.sync.dma_start(xb1[:], xb1_d[:])

            w1sb = wpool.tile([128, 2, CC], BF16, tag="w1sb")
            nc.sync.dma_start(w1sb[:, 0, :], w1t_d[0])
            nc.sync.dma_start(w1sb[:, 1, :], w1t_d[1])
            w2sb = wpool.tile([CC, 9, NM], BF16, tag="w2sb")
            nc.sync.dma_start(w2sb[:], w2t_d[:])
            b1c = wpool.tile([CC, 1], F32, tag="b1c")
            nc.sync.dma_start(b1c[:], b1_d[:])
            b2c = wpool.tile([NM, 1], F32, tag="b2c")
            nc.sync.dma_start(b2c[:], b2_d[:])
            osum = wpool.tile([NM, NQ], F32, tag="osum")
            nc.sync.dma_start(osum[:], osum_d[:])
            orep = wpool.tile([NQ, NM], F32, tag="orep")
            nc.sync.dma_start(orep[:], orep_d[:])

            i128 = wpool.tile([128, 128], BF16, tag="i128")
            nc.sync.dma_start(i128[:], i128_d[:])

            with tc.tile_pool(name="psum_head", bufs=2, space="PSUM") as psum:
                # ---- PE fences -----------------------------------------
                for fap in (
                    xb0[:, 0, 0:1], xb1[:, 0, 0:1], w1sb[:, 0, 0:1],
                    w2sb[:, 0, 0:1], osum[:, 0:1], orep[:, 0:1],
                    i128[:, 0:1],
                ):
                    psf = psum.tile([1, 1], F32, tag="psf")
                    nc.tensor.matmul(psf[:], fap, fap, start=True, stop=True)

                # ---- stage A: conv1x1 ----------------------------------
                xc = mpool.tile([CC, HP, WP], BF16, tag="xc")
                xc_flat = xc[:].rearrange("c h w -> c (h w)")
                x0_flat = xb0[:].rearrange("c h w -> c (h w)")
                x1_flat = xb1[:].rearrange("c h w -> c (h w)")
                CHUNK = 512
                nchunks = (NPADPIX + CHUNK - 1) // CHUNK
                for i in range(nchunks):
                    n0 = i * CHUNK
                    n1 = min(NPADPIX, n0 + CHUNK)
                    ps = psum.tile([CC, CHUNK], F32, tag="ps")
                    nc.tensor.matmul(
                        ps[:, : n1 - n0], w1sb[:, 0, :], x0_flat[:, n0:n1],
                        start=True, stop=False,
                    )
                    nc.tensor.matmul(
                        ps[:, : n1 - n0], w1sb[:, 1, :], x1_flat[:, n0:n1],
                        start=False, stop=True,
                    )
                    nc.vector.tensor_scalar_add(
                        xc_flat[:, n0:n1], ps[:, : n1 - n0], b1c[:, 0:1]
                    )

                # ---- stage B: conv3x3 -> exp ---------------------------
                msk_e = mpool.tile([NM, NPIX], F32, tag="msk_e")
                HROWS = 8
                for i in range(HL // HROWS):
                    psm = psum.tile([NM, HROWS, W], F32, tag="ps")
                    for tap in range(9):
                        dy, dx = tap // 3, tap % 3
                        rhs = xc[:, i * HROWS + 1 + dy : i * HROWS + 1 + dy + HROWS,
                                 1 + dx : 1 + dx + W]
                        nc.tensor.matmul(
                            psm[:], w2sb[:, tap, :], rhs,
                            start=(tap == 0), stop=(tap == 8),
                        )
                    me = msk_e[:].rearrange("m (h w) -> m h w", w=W)
                    nc.scalar.activation(
                        me[:, i * HROWS : (i + 1) * HROWS, :], psm[:],
                        mybir.ActivationFunctionType.Exp, bias=b2c[:, 0:1],
                    )

                # ---- stage C: normalize (bf16) + stage to DRAM ---------
                rs = mpool.tile([NQ, NPIX], F32, tag="rs")
                msk_nb = mpool.tile([NM, NPIX], BF16, tag="msk_nb")
                for i in range(NPIX // CHUNK):
                    pss = psum.tile([NQ, CHUNK], F32, tag="ps")
                    nc.tensor.matmul(
                        pss[:], osum[:], msk_e[:, i * CHUNK : (i + 1) * CHUNK],
                        start=True, stop=True,
                    )
                    nc.vector.reciprocal(
                        rs[:, i * CHUNK : (i + 1) * CHUNK], pss[:]
                    )
                    psr = psum.tile([NM, CHUNK], F32, tag="ps")
                    nc.tensor.matmul(
                        psr[:], orep[:], rs[:, i * CHUNK : (i + 1) * CHUNK],
                        start=True, stop=True,
                    )
                    nc.vector.tensor_mul(
                        msk_nb[:, i * CHUNK : (i + 1) * CHUNK],
                        msk_e[:, i * CHUNK : (i + 1) * CHUNK], psr[:],
                    )
                    nc.sync.dma_start(
                        msk_d[:, i * CHUNK : (i + 1) * CHUNK],
                        msk_nb[:, i * CHUNK : (i + 1) * CHUNK],
                    )

            # ---- stage D: combine --------------------------------------
            # DVE does every mul; adds split between DVE tensor_add and
            # PE identity-matmul accumulation into PSUM (4 banks per ch).
            CH4 = NPIX // 4  # 512, one PSUM bank
            with tc.tile_pool(name="psum_acc", bufs=1, space="PSUM") as psacc:
                xbs = (xb0, xb1)
                unit_idx = 0
                for q in range(NQ):
                    # precompute lane per (k, ch) for this q
                    lane = {}
                    pe_ks = {0: [], 1: []}
                    for k in range(KA):
                        for ch in range(2):
                            u = unit_idx
                            unit_idx += 1
                            use_dve = (u * _N_DVE_ADD) // 200 != (
                                (u + 1) * _N_DVE_ADD
                            ) // 200
                            lane[(k, ch)] = use_dve
                            if not use_dve:
                                pe_ks[ch].append(k)
                    acc_d = {}
                    acc_e = {}
                    for k in range(KA):
                        di, dj = k // K5, k % K5
                        row = q * KA + k
                        bc = bcpool.tile([128, NPIX], BF16, tag="bc")
                        nc.sync.dma_start(
                            bc[:],
                            msk_d[row : row + 1, :].broadcast_to((128, NPIX)),
                        )
                        bcv = bc[:].rearrange("p (h w) -> p h w", w=W)
                        for ch in range(2):
                            win = xbs[ch][:, di : di + HL, dj : dj + W]
                            if lane[(k, ch)]:
                                if ch not in acc_d:
                                    a = accpool.tile(
                                        [128, NPIX], BF16,
                                        tag=f"acc_d{ch}", bufs=2,
                                    )
                                    acc_d[ch] = a
                                    av = a[:].rearrange(
                                        "p (h w) -> p h w", w=W
                                    )
                                    nc.vector.tensor_mul(av, win, bcv)
                                else:
                                    a = acc_d[ch]
                                    pr = prpool.tile(
                                        [128, NPIX], BF16, tag="prd"
                                    )
                                    prv = pr[:].rearrange(
                                        "p (h w) -> p h w", w=W
                                    )
                                    nc.vector.tensor_mul(prv, win, bcv)
                                    nc.vector.tensor_add(a[:], a[:], pr[:])
                            else:
                                pr = prpool.tile(
                                    [128, NPIX], BF16, tag="prp"
                                )
                                prv = pr[:].rearrange("p (h w) -> p h w", w=W)
                                nc.vector.tensor_mul(prv, win, bcv)
                                first = k == pe_ks[ch][0]
                                last = k == pe_ks[ch][-1]
                                if first:
                                    acc_e[ch] = [
                                        psacc.tile(
                                            [128, CH4], F32,
                                            tag=f"pacc{ch}_{cc}",
                                            name=f"pacc{ch}_{cc}",
                                        )
                                        for cc in range(4)
                                    ]
                                for cc in range(4):
                                    nc.tensor.matmul(
                                        acc_e[ch][cc][:],
                                        i128[:],
                                        pr[:, cc * CH4 : (cc + 1) * CH4],
                                        start=first,
                                        stop=last,
                                    )
                    # ---- drain this q: copy PSUM partials, merge, DMA out
                    for ch in range(2):
                        parts = []
                        if ch in acc_e:
                            pe_sb = outpool.tile(
                                [128, NPIX], BF16, tag="pe_sb"
                            )
                            for cc in range(4):
                                nc.scalar.copy(
                                    pe_sb[:, cc * CH4 : (cc + 1) * CH4],
                                    acc_e[ch][cc][:],
                                )
                            parts.append(pe_sb)
                        if ch in acc_d:
                            parts.append(acc_d[ch])
                        if len(parts) == 2:
                            ost = outpool.tile([128, NPIX], BF16, tag="ost")
                            nc.vector.tensor_add(
                                ost[:], parts[0][:], parts[1][:]
                            )
                        else:
                            ost = parts[0]
                        nc.sync.dma_start(out_d[ch, :, q, :], ost[:])

    nc.compile()
    return nc


@lru_cache(maxsize=2)
def _get_program(trace_debug: bool = False):
    return _build_program()


def _host_prep(x, w1, b1, w2, b2):
    x = np.asarray(x, np.float32)
    w1 = np.asarray(w1, np.float32)
    b1 = np.asarray(b1, np.float32).reshape(CC, 1)
    w2 = np.asarray(w2, np.float32)
    b2 = np.asarray(b2, np.float32).reshape(NM, 1)

    w1t = np.ascontiguousarray(w1[:, :, 0, 0].T.reshape(2, 128, CC)).astype(_BF16NP)
    w2t = np.ascontiguousarray(
        w2.transpose(1, 2, 3, 0).reshape(CC, 9, NM)
    ).astype(_BF16NP)
    osum = np.zeros((NM, NQ), np.float32)
    for q in range(NQ):
        osum[q * KA : (q + 1) * KA, q] = 1.0
    orep = np.ascontiguousarray(osum.T)
    i128 = np.eye(128, dtype=np.float32).astype(_BF16NP)

    in_maps = []
    for s in range(N_CORES):
        b, hh = s // 2, s % 2
        h0 = hh * HL
        xpad = np.zeros((C, HP, WP), np.float32)
        r0 = max(0, h0 - 2)
        r1 = min(H, h0 + HL + 2)
        xpad[:, (r0 - h0 + 2) : (r1 - h0 + 2), 2 : 2 + W] = x[b, :, r0:r1, :]
        xb = xpad.astype(_BF16NP)
        in_maps.append(
            {
                "xb0": np.ascontiguousarray(xb[:128]),
                "xb1": np.ascontiguousarray(xb[128:]),
                "w1t": w1t,
                "w2t": w2t,
                "b1v": b1,
                "b2v": b2,
                "osum": osum,
                "orep": orep,
                "i128": i128,
            }
        )
    return in_maps


def _host_post(results):
    out = np.empty((B, C, H * SF, W * SF), np.float32)
    for s in range(N_CORES):
        b, hh = s // 2, s % 2
        o = np.asarray(results[s]["out"], np.float32)  # [2, 128, NQ, NPIX]
        o = o.reshape(2, 128, NQ, HL, W).reshape(C, SF, SF, HL, W)
        o = o.transpose(0, 3, 1, 4, 2).reshape(C, HL * SF, W * SF)
        out[b, :, hh * HL * SF : (hh + 1) * HL * SF, :] = o
    return out


def kernel(x, w1, b1, w2, b2):
    nc = _get_program(False)
    in_maps = _host_prep(x, w1, b1, w2, b2)
    res = run_bass_kernel_spmd(nc, in_maps, list(range(N_CORES)))
    return _host_post(res.results)


# revision 19
# speedup vs baseline: 1.9135x; 1.0150x over previous
"""CARAFE upsampling kernel for 8 Trainium2 NeuronCores.

Reference op (per batch b):
  xc   = conv1x1(x, w1) + b1                     # (CC=64, H, W)
  mask = conv3x3(xc, w2, pad=1) + b2             # (100, H, W)
  mask = softmax over the 25 kernel taps (per q in 4 = SF*SF groups)
  out[q, c, h, w] = sum_k mask[q, k, h, w] * x[c, h+di-2, w+dj-2]
  out pixel-shuffled by SF=2 -> (C, 2H, 2W)

Sharding: 8 shards = batch(4) x H-halves(2).

Combine strategy (channel-major, wide bf16 ops in DVE 2x mode): per
(tap k, quadrant q) the normalized mask row [2048 px] is partition-
broadcast to a [128, 2048] bf16 tile via a stride-0 DRAM-source DMA
(runs on the DMA engines, off the compute path). The 25-tap x 4q x 2ch
accumulation then runs as [128, 2048] ops on balanced lanes:
  - PAIR: DVE tensor_mul + DVE tensor_add (both bf16 2x, ~1.2 us each)
  - XPOOL: DVE tensor_mul -> GPSIMD tensor_add (~4.2 us)
Each (q, ch) keeps one accumulator per adding engine; partials are
merged on DVE at the end and written out in bf16.
"""

import os
from functools import lru_cache

import numpy as np

import concourse.bass as bass
import concourse.mybir as mybir
from concourse import bacc
import concourse.tile as tile
from concourse.bass_utils import run_bass_kernel_spmd

F32 = mybir.dt.float32
BF16 = mybir.dt.bfloat16
import ml_dtypes as _mld

_BF16NP = _mld.bfloat16

B, C, H, W = 4, 256, 64, 64
CC = 64
SF = 2
K5 = 5
KA = K5 * K5
NQ = SF * SF
NM = NQ * KA

HL = 32
HP = HL + 4
WP = W + 4
NPIX = HL * W
NPADPIX = HP * WP

N_CORES = 8

# measured per-[128,2048]-op engine costs (ns) for lane balancing
_DVE_MUL = 1250.0
_DVE_ADD = 1210.0
# PE identity-matmul add: 4x ([128,512] matmul + ldweights) per unit
_PE_ADD = 2300.0
# DVE-add units out of 200 (rest accumulate on PE via PSUM)
_N_DVE_ADD = 88


def _build_program():
    nc = bacc.Bacc("TRN2", target_bir_lowering=False, debug=False)

    xb0_d = nc.dram_tensor("xb0", [128, HP, WP], BF16, kind="ExternalInput")
    xb1_d = nc.dram_tensor("xb1", [128, HP, WP], BF16, kind="ExternalInput")
    w1t_d = nc.dram_tensor("w1t", [2, 128, CC], BF16, kind="ExternalInput")
    w2t_d = nc.dram_tensor("w2t", [CC, 9, NM], BF16, kind="ExternalInput")
    b1_d = nc.dram_tensor("b1v", [CC, 1], F32, kind="ExternalInput")
    b2_d = nc.dram_tensor("b2v", [NM, 1], F32, kind="ExternalInput")
    osum_d = nc.dram_tensor("osum", [NM, NQ], F32, kind="ExternalInput")
    orep_d = nc.dram_tensor("orep", [NQ, NM], F32, kind="ExternalInput")
    i128_d = nc.dram_tensor("i128", [128, 128], BF16, kind="ExternalInput")
    # normalized-mask staging in DRAM for stride-0 broadcast reads
    msk_d = nc.dram_tensor("mskd", [NM, NPIX], BF16, kind="Internal")

    out_d = nc.dram_tensor("out", [2, 128, NQ, NPIX], BF16, kind="ExternalOutput")

    with tile.TileContext(nc) as tc:
        with (
            tc.tile_pool(name="xpool", bufs=1) as xpool,
            tc.tile_pool(name="wpool", bufs=1) as wpool,
            tc.tile_pool(name="mpool", bufs=1) as mpool,
            tc.tile_pool(name="acc", bufs=1) as accpool,
            tc.tile_pool(name="bcast", bufs=6) as bcpool,
            tc.tile_pool(name="prod", bufs=6) as prpool,
            tc.tile_pool(name="ostage", bufs=4) as outpool,
        ):
            # ---- load inputs -------------------------------------------
            xb0 = xpool.tile([128, HP, WP], BF16, tag="xb0")
            xb1 = xpool.tile([128, HP, WP], BF16, tag="xb1")
            nc.sync.dma_start(xb0[:], xb0_d[:])
            nc.sync.dma_start(xb1[:], xb1_d[:])

            w1sb = wpool.tile([128, 2, CC], BF16, tag="w1sb")
            nc.sync.dma_start(w1sb[:, 0, :], w1t_d[0])
            nc.sync.dma_start(w1sb[:, 1, :], w1t_d[1])
            w2sb = wpool.tile([CC, 9, NM], BF16, tag="w2sb")
            nc.sync.dma_start(w2sb[:], w2t_d[:])
            b1c = wpool.tile([CC, 1], F32, tag="b1c")
            nc.sync.dma_start(b1c[:], b1_d[:])
            b2c = wpool.tile([NM, 1], F32, tag="b2c")
            nc.sync.dma_start(b2c[:], b2_d[:])
            osum = wpool.tile([NM, NQ], F32, tag="osum")
            nc.sync.dma_start(osum[:], osum_d[:])
            orep = wpool.tile([NQ, NM], F32, tag="orep")
            nc.sync.dma_start(orep[:], orep_d[:])

            i128 = wpool.tile([128, 128], BF16, tag="i128")
            nc.sync.dma_start(i128[:], i128_d[:])

            with tc.tile_pool(name="psum_head", bufs=2, space="PSUM") as psum:
                # ---- PE fences -----------------------------------------
                for fap in (
                    xb0[:, 0, 0:1], xb1[:, 0, 0:1], w1sb[:, 0, 0:1],
                    w2sb[:, 0, 0:1], osum[:, 0:1], orep[:, 0:1],
                    i128[:, 0:1],
                ):
                    psf = psum.tile([1, 1], F32, tag="psf")
                    nc.tensor.matmul(psf[:], fap, fap, start=True, stop=True)

                # ---- stage A: conv1x1 ----------------------------------
                xc = mpool.tile([CC, HP, WP], BF16, tag="xc")
                xc_flat = xc[:].rearrange("c h w -> c (h w)")
                x0_flat = xb0[:].rearrange("c h w -> c (h w)")
                x1_flat = xb1[:].rearrange("c h w -> c (h w)")
                CHUNK = 512
                nchunks = (NPADPIX + CHUNK - 1) // CHUNK
                for i in range(nchunks):
                    n0 = i * CHUNK
                    n1 = min(NPADPIX, n0 + CHUNK)
                    ps = psum.tile([CC, CHUNK], F32, tag="ps")
                    nc.tensor.matmul(
                        ps[:, : n1 - n0], w1sb[:, 0, :], x0_flat[:, n0:n1],
                        start=True, stop=False,
                    )
                    nc.tensor.matmul(
                        ps[:, : n1 - n0], w1sb[:, 1, :], x1_flat[:, n0:n1],
                        start=False, stop=True,
                    )
                    nc.vector.tensor_scalar_add(
                        xc_flat[:, n0:n1], ps[:, : n1 - n0], b1c[:, 0:1]
                    )

                # ---- stage B: conv3x3 -> exp ---------------------------
                msk_e = mpool.tile([NM, NPIX], F32, tag="msk_e")
                HROWS = 8
                for i in range(HL // HROWS):
                    psm = psum.tile([NM, HROWS, W], F32, tag="ps")
                    for tap in range(9):
                        dy, dx = tap // 3, tap % 3
                        rhs = xc[:, i * HROWS + 1 + dy : i * HROWS + 1 + dy + HROWS,
                                 1 + dx : 1 + dx + W]
                        nc.tensor.matmul(
                            psm[:], w2sb[:, tap, :], rhs,
                            start=(tap == 0), stop=(tap == 8),
                        )
                    me = msk_e[:].rearrange("m (h w) -> m h w", w=W)
                    nc.scalar.activation(
                        me[:, i * HROWS : (i + 1) * HROWS, :], psm[:],
                        mybir.ActivationFunctionType.Exp, bias=b2c[:, 0:1],
                    )

                # ---- stage C: normalize (bf16) + stage to DRAM ---------
                rs = mpool.tile([NQ, NPIX], F32, tag="rs")
                msk_nb = mpool.tile([NM, NPIX], BF16, tag="msk_nb")
                for i in range(NPIX // CHUNK):
                    pss = psum.tile([NQ, CHUNK], F32, tag="ps")
                    nc.tensor.matmul(
                        pss[:], osum[:], msk_e[:, i * CHUNK : (i + 1) * CHUNK],
                        start=True, stop=True,
                    )
                    nc.vector.reciprocal(
                        rs[:, i * CHUNK : (i + 1) * CHUNK], pss[:]
                    )
                    psr = psum.tile([NM, CHUNK], F32, tag="ps")
                    nc.tensor.matmul(
                        psr[:], orep[:], rs[:, i * CHUNK : (i + 1) * CHUNK],
                        start=True, stop=True,
                    )
                    nc.vector.tensor_mul(
                        msk_nb[:, i * CHUNK : (i + 1) * CHUNK],
                        msk_e[:, i * CHUNK : (i + 1) * CHUNK], psr[:],
                    )
                    nc.sync.dma_start(
                        msk_d[:, i * CHUNK : (i + 1) * CHUNK],
                        msk_nb[:, i * CHUNK : (i + 1) * CHUNK],
                    )

            # ---- stage D: combine --------------------------------------
            # DVE does every mul; adds split between DVE tensor_add and
            # PE identity-matmul accumulation into PSUM (4 banks per ch).
            CH4 = NPIX // 4  # 512, one PSUM bank
            with tc.tile_pool(name="psum_acc", bufs=1, space="PSUM") as psacc:
                xbs = (xb0, xb1)
                unit_idx = 0
                for q in range(NQ):
                    # precompute lane per (k, ch) for this q
                    lane = {}
                    pe_ks = {0: [], 1: []}
                    for k in range(KA):
                        for ch in range(2):
                            u = unit_idx
                            unit_idx += 1
                            use_dve = (u * _N_DVE_ADD) // 200 != (
                                (u + 1) * _N_DVE_ADD
                            ) // 200
                            lane[(k, ch)] = use_dve
                            if not use_dve:
                                pe_ks[ch].append(k)
                    acc_d = {}
                    acc_e = {}
                    for k in range(KA):
                        di, dj = k // K5, k % K5
                        row = q * KA + k
                        bc = bcpool.tile([128, NPIX], BF16, tag="bc")
                        nc.sync.dma_start(
                            bc[:],
                            msk_d[row : row + 1, :].broadcast_to((128, NPIX)),
                        )
                        bcv = bc[:].rearrange("p (h w) -> p h w", w=W)
                        for ch in range(2):
                            win = xbs[ch][:, di : di + HL, dj : dj + W]
                            if lane[(k, ch)]:
                                if ch not in acc_d:
                                    a = accpool.tile(
                                        [128, NPIX], BF16,
                                        tag=f"acc_d{ch}", bufs=2,
                                    )
                                    acc_d[ch] = a
                                    av = a[:].rearrange(
                                        "p (h w) -> p h w", w=W
                                    )
                                    nc.vector.tensor_mul(av, win, bcv)
                                else:
                                    a = acc_d[ch]
                                    pr = prpool.tile(
                                        [128, NPIX], BF16, tag="prd"
                                    )
                                    prv = pr[:].rearrange(
                                        "p (h w) -> p h w", w=W
                                    )
                                    nc.vector.tensor_mul(prv, win, bcv)
                                    nc.vector.tensor_add(a[:], a[:], pr[:])
                            else:
                                pr = prpool.tile(
                                    [128, NPIX], BF16, tag="prp"
                                )
                                prv = pr[:].rearrange("p (h w) -> p h w", w=W)
                                nc.vector.tensor_mul(prv, win, bcv)
                                first = k == pe_ks[ch][0]
                                last = k == pe_ks[ch][-1]
                                if first:
                                    acc_e[ch] = [
                                        psacc.tile(
                                            [128, CH4], F32,
                                            tag=f"pacc{ch}_{cc}",
                                            name=f"pacc{ch}_{cc}",
                                        )
                                        for cc in range(4)
                                    ]
                                for cc in range(4):
                                    nc.tensor.matmul(
                                        acc_e[ch][cc][:],
                                        i128[:],
                                        pr[:, cc * CH4 : (cc + 1) * CH4],
                                        start=first,
                                        stop=last,
                                    )
                    # ---- drain this q: copy PSUM partials, merge, DMA out
                    for ch in range(2):
                        parts = []
                        if ch in acc_e:
                            pe_sb = outpool.tile(
                                [128, NPIX], BF16, tag="pe_sb"
                            )
                            for cc in range(4):
                                nc.scalar.copy(
                                    pe_sb[:, cc * CH4 : (cc + 1) * CH4],
                                    acc_e[ch][cc][:],
                                )
                            parts.append(pe_sb)
                        if ch in acc_d:
                            parts.append(acc_d[ch])
                        if len(parts) == 2:
                            ost = outpool.tile([128, NPIX], BF16, tag="ost")
                            nc.vector.tensor_add(
                                ost[:], parts[0][:], parts[1][:]
                            )
                        else:
                            ost = parts[0]
                        nc.sync.dma_start(out_d[ch, :, q, :], ost[:])

    nc.compile()
    return nc


@lru_cache(maxsize=2)
def _get_program(trace_debug: bool = False):
    return _build_program()


def _host_prep(x, w1, b1, w2, b2):
    x = np.asarray(x, np.float32)
    w1 = np.asarray(w1, np.float32)
    b1 = np.asarray(b1, np.float32).reshape(CC, 1)
    w2 = np.asarray(w2, np.float32)
    b2 = np.asarray(b2, np.float32).reshape(NM, 1)

    w1t = np.ascontiguousarray(w1[:, :, 0, 0].T.reshape(2, 128, CC)).astype(_BF16NP)
    w2t = np.ascontiguousarray(
        w2.transpose(1, 2, 3, 0).reshape(CC, 9, NM)
    ).astype(_BF16NP)
    osum = np.zeros((NM, NQ), np.float32)
    for q in range(NQ):
        osum[q * KA : (q + 1) * KA, q] = 1.0
    orep = np.ascontiguousarray(osum.T)
    i128 = np.eye(128, dtype=np.float32).astype(_BF16NP)

    in_maps = []
    for s in range(N_CORES):
        b, hh = s // 2, s % 2
        h0 = hh * HL
        xpad = np.zeros((C, HP, WP), np.float32)
        r0 = max(0, h0 - 2)
        r1 = min(H, h0 + HL + 2)
        xpad[:, (r0 - h0 + 2) : (r1 - h0 + 2), 2 : 2 + W] = x[b, :, r0:r1, :]
        xb = xpad.astype(_BF16NP)
        in_maps.append(
            {
                "xb0": np.ascontiguousarray(xb[:128]),
                "xb1": np.ascontiguousarray(xb[128:]),
                "w1t": w1t,
                "w2t": w2t,
                "b1v": b1,
                "b2v": b2,
                "osum": osum,
                "orep": orep,
                "i128": i128,
            }
        )
    return in_maps


def _host_post(results):
    out = np.empty((B, C, H * SF, W * SF), np.float32)
    for s in range(N_CORES):
        b, hh = s // 2, s % 2
        o = np.asarray(results[s]["out"], np.float32)  # [2, 128, NQ, NPIX]
        o = o.reshape(2, 128, NQ, HL, W).reshape(C, SF, SF, HL, W)
        o = o.transpose(0, 3, 1, 4, 2).reshape(C, HL * SF, W * SF)
        out[b, :, hh * HL * SF : (hh + 1) * HL * SF, :] = o
    return out


def kernel(x, w1, b1, w2, b2):
    nc = _get_program(False)
    in_maps = _host_prep(x, w1, b1, w2, b2)
    res = run_bass_kernel_spmd(nc, in_maps, list(range(N_CORES)))
    return _host_post(res.results)


# revision 22
# speedup vs baseline: 1.9204x; 1.0036x over previous
"""CARAFE upsampling kernel for 8 Trainium2 NeuronCores.

Reference op (per batch b):
  xc   = conv1x1(x, w1) + b1                     # (CC=64, H, W)
  mask = conv3x3(xc, w2, pad=1) + b2             # (100, H, W)
  mask = softmax over the 25 kernel taps (per q in 4 = SF*SF groups)
  out[q, c, h, w] = sum_k mask[q, k, h, w] * x[c, h+di-2, w+dj-2]
  out pixel-shuffled by SF=2 -> (C, 2H, 2W)

Sharding: 8 shards = batch(4) x H-halves(2).

Combine strategy (channel-major, wide bf16 ops in DVE 2x mode): per
(tap k, quadrant q) the normalized mask row [2048 px] is partition-
broadcast to a [128, 2048] bf16 tile via a stride-0 DRAM-source DMA
(runs on the DMA engines, off the compute path). DVE computes every
product (tensor_mul, bf16 2x, ~1.2 us per [128, 2048] tile); the
25-tap accumulation is split between DVE tensor_add (88 of 200 units)
and PE identity-matmul accumulation into PSUM (112 units, fp32 banks)
so the two engines finish together. GPSIMD is deliberately idle: its
SBUF port is shared with DVE and concurrent Pool tensor ops knock DVE
out of 2x mode (measured 1.2 -> 3.1 us). Per q, the PSUM partials are
copied to SBUF on the Scalar engine, merged with the DVE partial, and
written out in bf16; the host un-shuffles pixels and casts to f32.
"""

import os
from functools import lru_cache

import numpy as np

import concourse.bass as bass
import concourse.mybir as mybir
from concourse import bacc
import concourse.tile as tile
from concourse.bass_utils import run_bass_kernel_spmd

F32 = mybir.dt.float32
BF16 = mybir.dt.bfloat16
import ml_dtypes as _mld

_BF16NP = _mld.bfloat16

B, C, H, W = 4, 256, 64, 64
CC = 64
SF = 2
K5 = 5
KA = K5 * K5
NQ = SF * SF
NM = NQ * KA

HL = 32
HP = HL + 4
WP = W + 4
NPIX = HL * W
NPADPIX = HP * WP

N_CORES = 8

# measured per-[128,2048]-op engine costs (ns) for lane balancing
_DVE_MUL = 1250.0
_DVE_ADD = 1210.0
# PE identity-matmul add: 4x ([128,512] matmul + ldweights) per unit
_PE_ADD = 2300.0
# DVE-add units out of 200 (rest accumulate on PE via PSUM)
_N_DVE_ADD = 85


def _build_program():
    nc = bacc.Bacc("TRN2", target_bir_lowering=False, debug=False)

    xb0_d = nc.dram_tensor("xb0", [128, HP, WP], BF16, kind="ExternalInput")
    xb1_d = nc.dram_tensor("xb1", [128, HP, WP], BF16, kind="ExternalInput")
    w1t_d = nc.dram_tensor("w1t", [2, 128, CC], BF16, kind="ExternalInput")
    w2t_d = nc.dram_tensor("w2t", [CC, 9, NM], BF16, kind="ExternalInput")
    b1_d = nc.dram_tensor("b1v", [CC, 1], F32, kind="ExternalInput")
    b2_d = nc.dram_tensor("b2v", [NM, 1], F32, kind="ExternalInput")
    osum_d = nc.dram_tensor("osum", [NM, NQ], F32, kind="ExternalInput")
    orep_d = nc.dram_tensor("orep", [NQ, NM], F32, kind="ExternalInput")
    i128_d = nc.dram_tensor("i128", [128, 128], BF16, kind="ExternalInput")
    # normalized-mask staging in DRAM for stride-0 broadcast reads
    msk_d = nc.dram_tensor("mskd", [NM, NPIX], BF16, kind="Internal")

    out_d = nc.dram_tensor("out", [2, 128, NQ, NPIX], BF16, kind="ExternalOutput")

    with tile.TileContext(nc) as tc:
        with (
            tc.tile_pool(name="xpool", bufs=1) as xpool,
            tc.tile_pool(name="wpool", bufs=1) as wpool,
            tc.tile_pool(name="mpool", bufs=1) as mpool,
            tc.tile_pool(name="acc", bufs=1) as accpool,
            tc.tile_pool(name="bcast", bufs=6) as bcpool,
            tc.tile_pool(name="prod", bufs=6) as prpool,
            tc.tile_pool(name="ostage", bufs=4) as outpool,
        ):
            # ---- load inputs -------------------------------------------
            xb0 = xpool.tile([128, HP, WP], BF16, tag="xb0")
            xb1 = xpool.tile([128, HP, WP], BF16, tag="xb1")
            nc.sync.dma_start(xb0[:], xb0_d[:])
            nc.sync.dma_start(xb1[:], xb1_d[:])

            w1sb = wpool.tile([128, 2, CC], BF16, tag="w1sb")
            nc.sync.dma_start(w1sb[:, 0, :], w1t_d[0])
            nc.sync.dma_start(w1sb[:, 1, :], w1t_d[1])
            w2sb = wpool.tile([CC, 9, NM], BF16, tag="w2sb")
            nc.sync.dma_start(w2sb[:], w2t_d[:])
            b1c = wpool.tile([CC, 1], F32, tag="b1c")
            nc.sync.dma_start(b1c[:], b1_d[:])
            b2c = wpool.tile([NM, 1], F32, tag="b2c")
            nc.sync.dma_start(b2c[:], b2_d[:])
            osum = wpool.tile([NM, NQ], F32, tag="osum")
            nc.sync.dma_start(osum[:], osum_d[:])
            orep = wpool.tile([NQ, NM], F32, tag="orep")
            nc.sync.dma_start(orep[:], orep_d[:])

            i128 = wpool.tile([128, 128], BF16, tag="i128")
            nc.sync.dma_start(i128[:], i128_d[:])

            with tc.tile_pool(name="psum_head", bufs=2, space="PSUM") as psum:
                # ---- PE fences -----------------------------------------
                for fap in (
                    xb0[:, 0, 0:1], xb1[:, 0, 0:1], w1sb[:, 0, 0:1],
                    w2sb[:, 0, 0:1], osum[:, 0:1], orep[:, 0:1],
                    i128[:, 0:1],
                ):
                    psf = psum.tile([1, 1], F32, tag="psf")
                    nc.tensor.matmul(psf[:], fap, fap, start=True, stop=True)

                # ---- stage A: conv1x1 ----------------------------------
                xc = mpool.tile([CC, HP, WP], BF16, tag="xc")
                xc_flat = xc[:].rearrange("c h w -> c (h w)")
                x0_flat = xb0[:].rearrange("c h w -> c (h w)")
                x1_flat = xb1[:].rearrange("c h w -> c (h w)")
                CHUNK = 512
                nchunks = (NPADPIX + CHUNK - 1) // CHUNK
                for i in range(nchunks):
                    n0 = i * CHUNK
                    n1 = min(NPADPIX, n0 + CHUNK)
                    ps = psum.tile([CC, CHUNK], F32, tag="ps")
                    nc.tensor.matmul(
                        ps[:, : n1 - n0], w1sb[:, 0, :], x0_flat[:, n0:n1],
                        start=True, stop=False,
                    )
                    nc.tensor.matmul(
                        ps[:, : n1 - n0], w1sb[:, 1, :], x1_flat[:, n0:n1],
                        start=False, stop=True,
                    )
                    nc.vector.tensor_scalar_add(
                        xc_flat[:, n0:n1], ps[:, : n1 - n0], b1c[:, 0:1]
                    )

                # ---- stage B: conv3x3 -> exp ---------------------------
                msk_e = mpool.tile([NM, NPIX], F32, tag="msk_e")
                HROWS = 8
                for i in range(HL // HROWS):
                    psm = psum.tile([NM, HROWS, W], F32, tag="ps")
                    for tap in range(9):
                        dy, dx = tap // 3, tap % 3
                        rhs = xc[:, i * HROWS + 1 + dy : i * HROWS + 1 + dy + HROWS,
                                 1 + dx : 1 + dx + W]
                        nc.tensor.matmul(
                            psm[:], w2sb[:, tap, :], rhs,
                            start=(tap == 0), stop=(tap == 8),
                        )
                    me = msk_e[:].rearrange("m (h w) -> m h w", w=W)
                    nc.scalar.activation(
                        me[:, i * HROWS : (i + 1) * HROWS, :], psm[:],
                        mybir.ActivationFunctionType.Exp, bias=b2c[:, 0:1],
                    )

                # ---- stage C: normalize (bf16) + stage to DRAM ---------
                rs = mpool.tile([NQ, NPIX], F32, tag="rs")
                msk_nb = mpool.tile([NM, NPIX], BF16, tag="msk_nb")
                for i in range(NPIX // CHUNK):
                    pss = psum.tile([NQ, CHUNK], F32, tag="ps")
                    nc.tensor.matmul(
                        pss[:], osum[:], msk_e[:, i * CHUNK : (i + 1) * CHUNK],
                        start=True, stop=True,
                    )
                    nc.vector.reciprocal(
                        rs[:, i * CHUNK : (i + 1) * CHUNK], pss[:]
                    )
                    psr = psum.tile([NM, CHUNK], F32, tag="ps")
                    nc.tensor.matmul(
                        psr[:], orep[:], rs[:, i * CHUNK : (i + 1) * CHUNK],
                        start=True, stop=True,
                    )
                    nc.vector.tensor_mul(
                        msk_nb[:, i * CHUNK : (i + 1) * CHUNK],
                        msk_e[:, i * CHUNK : (i + 1) * CHUNK], psr[:],
                    )
                    nc.sync.dma_start(
                        msk_d[:, i * CHUNK : (i + 1) * CHUNK],
                        msk_nb[:, i * CHUNK : (i + 1) * CHUNK],
                    )

            # ---- stage D: combine --------------------------------------
            # DVE does every mul; adds split between DVE tensor_add and
            # PE identity-matmul accumulation into PSUM (4 banks per ch).
            CH4 = NPIX // 4  # 512, one PSUM bank
            with tc.tile_pool(name="psum_acc", bufs=1, space="PSUM") as psacc:
                xbs = (xb0, xb1)
                unit_idx = 0
                for q in range(NQ):
                    # precompute lane per (k, ch) for this q
                    lane = {}
                    pe_ks = {0: [], 1: []}
                    for k in range(KA):
                        for ch in range(2):
                            u = unit_idx
                            unit_idx += 1
                            use_dve = (u * _N_DVE_ADD) // 200 != (
                                (u + 1) * _N_DVE_ADD
                            ) // 200
                            lane[(k, ch)] = use_dve
                            if not use_dve:
                                pe_ks[ch].append(k)
                    acc_d = {}
                    acc_e = {}
                    for k in range(KA):
                        di, dj = k // K5, k % K5
                        row = q * KA + k
                        bc = bcpool.tile([128, NPIX], BF16, tag="bc")
                        nc.sync.dma_start(
                            bc[:],
                            msk_d[row : row + 1, :].broadcast_to((128, NPIX)),
                        )
                        bcv = bc[:].rearrange("p (h w) -> p h w", w=W)
                        for ch in range(2):
                            win = xbs[ch][:, di : di + HL, dj : dj + W]
                            if lane[(k, ch)]:
                                if ch not in acc_d:
                                    a = accpool.tile(
                                        [128, NPIX], BF16,
                                        tag=f"acc_d{ch}", bufs=2,
                                    )
                                    acc_d[ch] = a
                                    av = a[:].rearrange(
                                        "p (h w) -> p h w", w=W
                                    )
                                    nc.vector.tensor_mul(av, win, bcv)
                                else:
                                    a = acc_d[ch]
                                    pr = prpool.tile(
                                        [128, NPIX], BF16, tag="prd"
                                    )
                                    prv = pr[:].rearrange(
                                        "p (h w) -> p h w", w=W
                                    )
                                    nc.vector.tensor_mul(prv, win, bcv)
                                    nc.vector.tensor_add(a[:], a[:], pr[:])
                            else:
                                pr = prpool.tile(
                                    [128, NPIX], BF16, tag="prp"
                                )
                                prv = pr[:].rearrange("p (h w) -> p h w", w=W)
                                nc.vector.tensor_mul(prv, win, bcv)
                                first = k == pe_ks[ch][0]
                                last = k == pe_ks[ch][-1]
                                if first:
                                    acc_e[ch] = [
                                        psacc.tile(
                                            [128, CH4], F32,
                                            tag=f"pacc{ch}_{cc}",
                                            name=f"pacc{ch}_{cc}",
                                        )
                                        for cc in range(4)
                                    ]
                                for cc in range(4):
                                    nc.tensor.matmul(
                                        acc_e[ch][cc][:],
                                        i128[:],
                                        pr[:, cc * CH4 : (cc + 1) * CH4],
                                        start=first,
                                        stop=last,
                                    )
                    # ---- drain this q: copy PSUM partials, merge, DMA out
                    for ch in range(2):
                        parts = []
                        if ch in acc_e:
                            pe_sb = outpool.tile(
                                [128, NPIX], BF16, tag="pe_sb"
                            )
                            for cc in range(4):
                                nc.scalar.copy(
                                    pe_sb[:, cc * CH4 : (cc + 1) * CH4],
                                    acc_e[ch][cc][:],
                                )
                            parts.append(pe_sb)
                        if ch in acc_d:
                            parts.append(acc_d[ch])
                        if len(parts) == 2:
                            ost = outpool.tile([128, NPIX], BF16, tag="ost")
                            nc.vector.tensor_add(
                                ost[:], parts[0][:], parts[1][:]
                            )
                        else:
                            ost = parts[0]
                        nc.sync.dma_start(out_d[ch, :, q, :], ost[:])

    nc.compile()
    return nc


@lru_cache(maxsize=2)
def _get_program(trace_debug: bool = False):
    return _build_program()


def _host_prep(x, w1, b1, w2, b2):
    x = np.asarray(x, np.float32)
    w1 = np.asarray(w1, np.float32)
    b1 = np.asarray(b1, np.float32).reshape(CC, 1)
    w2 = np.asarray(w2, np.float32)
    b2 = np.asarray(b2, np.float32).reshape(NM, 1)

    w1t = np.ascontiguousarray(w1[:, :, 0, 0].T.reshape(2, 128, CC)).astype(_BF16NP)
    w2t = np.ascontiguousarray(
        w2.transpose(1, 2, 3, 0).reshape(CC, 9, NM)
    ).astype(_BF16NP)
    osum = np.zeros((NM, NQ), np.float32)
    for q in range(NQ):
        osum[q * KA : (q + 1) * KA, q] = 1.0
    orep = np.ascontiguousarray(osum.T)
    i128 = np.eye(128, dtype=np.float32).astype(_BF16NP)

    in_maps = []
    for s in range(N_CORES):
        b, hh = s // 2, s % 2
        h0 = hh * HL
        xpad = np.zeros((C, HP, WP), np.float32)
        r0 = max(0, h0 - 2)
        r1 = min(H, h0 + HL + 2)
        xpad[:, (r0 - h0 + 2) : (r1 - h0 + 2), 2 : 2 + W] = x[b, :, r0:r1, :]
        xb = xpad.astype(_BF16NP)
        in_maps.append(
            {
                "xb0": np.ascontiguousarray(xb[:128]),
                "xb1": np.ascontiguousarray(xb[128:]),
                "w1t": w1t,
                "w2t": w2t,
                "b1v": b1,
                "b2v": b2,
                "osum": osum,
                "orep": orep,
                "i128": i128,
            }
        )
    return in_maps


def _host_post(results):
    out = np.empty((B, C, H * SF, W * SF), np.float32)
    for s in range(N_CORES):
        b, hh = s // 2, s % 2
        o = np.asarray(results[s]["out"], np.float32)  # [2, 128, NQ, NPIX]
        o = o.reshape(2, 128, NQ, HL, W).reshape(C, SF, SF, HL, W)
        o = o.transpose(0, 3, 1, 4, 2).reshape(C, HL * SF, W * SF)
        out[b, :, hh * HL * SF : (hh + 1) * HL * SF, :] = o
    return out


def kernel(x, w1, b1, w2, b2):
    nc = _get_program(False)
    in_maps = _host_prep(x, w1, b1, w2, b2)
    res = run_bass_kernel_spmd(nc, in_maps, list(range(N_CORES)))
    return _host_post(res.results)
